# revision 16
# baseline (speedup 1.0000x reference)
"""CosArcLoss on 8 TRN2 NeuronCores (Bass/Tile), fp8 two-region pipeline.

Math (reference, f32):
    t_i   = preds[i, labels[i]]
    num_i = 30*(cos(arccos(clip(t_i)) + 0.5) - 0.35)
    S_i   = sum_{j != labels[i]} exp(30*preds[i,j])
    loss  = mean_i( log(exp(num_i) + S_i) - num_i )

Device does all O(B*V) work: sum_j exp(30*q(x_ij)) over fp8-quantized
inputs (tolerance 2e-2 >> fp8 logsumexp bias ~4e-3). Host does the O(B)
epilogue: numerator from exact f32 targets, subtraction of the (exactly
simulated) target-column device contribution, final log/mean.

Per-core layout (256 rows, 32000 classes), split by class:
  region A (classes [0, CA)):  row-major fp8, HWDGE DMA, ScalarE exp
      (scale=30) with free accum_out row-sums. ~0.83 ns/elem on ACT.
      Two small warm-up tiles per row group start the ACT pipeline early.
  region B (classes [CA, V)):  transposed fp8 [class, row] chunks of 128
      classes, SWDGE DMA casts fp8->bf16 in flight (~305 GB/s write-side,
      queue-serialized so few big DMAs), VectorE computes exp via the
      Schraudolph exp2 bit-trick (i16 = rint(x*S1+S2) whose bits ARE
      bf16(exp(30x)), 4x mode ~0.27 ns/elem), TensorE ones-matmul reduces
      pairs of chunks (N=512 = one PSUM bank) into 4 rotating accumulators.
All engines stream concurrently; the split CA/VB balances ACT busy time
against the SWDGE cast-queue drain rate.
"""
import numpy as np
import ml_dtypes
from contextlib import ExitStack

import concourse.bass as bass
import concourse.tile as tile
from concourse import bacc, mybir
from concourse.bass_utils import run_bass_kernel_spmd

B, V = 2048, 32000
N_CORES = 8
RPC = B // N_CORES            # 256 rows per core
P = 128                       # SBUF partitions
G = RPC // P                  # 2 row groups (region A)

CA = 16384                    # classes handled by ACT (region A)
VB = V - CA                   # classes handled by DVE+TensorE (region B)
NCH = VB // P                 # 122 chunks of 128 classes
ATILES = [1024, 3072, 4096, 4096, 4096]  # per-group ACT tile widths
NTA = len(ATILES)
assert sum(ATILES) == CA
KCH = 8                       # chunks per B DMA / DVE schraudolph op
NBANK = 4                     # rotating PSUM accumulators (512 cols each)

SCALE = 30.0
LN2 = float(np.log(2.0))
S1 = 128.0 * SCALE / LN2           # schraudolph slope (bf16 bits / x)
C0 = 0.0564005                     # zero-mean-rel-err offset
S2 = 128.0 * (127.0 - C0)

F32 = mybir.dt.float32
BF16 = mybir.dt.bfloat16
I16 = mybir.dt.int16
FP8 = mybir.dt.float8e4
AF = mybir.ActivationFunctionType
ALU = mybir.AluOpType
E4M3 = ml_dtypes.float8_e4m3

_cache = {}


def _build():
    nc = bacc.Bacc("TRN2", target_bir_lowering=False, debug=False,
                   num_devices=N_CORES)
    xa = nc.dram_tensor("xa", [RPC, CA], FP8, kind="ExternalInput")
    xbt = nc.dram_tensor("xbt", [P, NCH * RPC], BF16, kind="ExternalInput")
    osa = nc.dram_tensor("osa", [P, G * NTA], F32, kind="ExternalOutput")
    osb = nc.dram_tensor("osb", [1, NBANK * 2 * RPC], F32,
                         kind="ExternalOutput")

    with tile.TileContext(nc) as tc, ExitStack() as ctx:
        apool = ctx.enter_context(tc.tile_pool(name="ap", bufs=12))
        bpool = ctx.enter_context(tc.tile_pool(name="bp", bufs=4))
        epool = ctx.enter_context(tc.tile_pool(name="ep", bufs=2))
        ipool = ctx.enter_context(tc.tile_pool(name="ip", bufs=4))
        spool = ctx.enter_context(tc.tile_pool(name="sp", bufs=1))
        psum = ctx.enter_context(tc.tile_pool(name="ps", bufs=1, space="PSUM"))

        ssum = spool.tile([P, G * NTA], F32)
        ones = spool.tile([P, 1], BF16)
        nc.any.memset(ones[:], 1.0)
        banks = [psum.tile([P, 2 * RPC], F32, name=f"bank{k}")
                 for k in range(NBANK)]

        # ---- single HWDGE queue, issue order = bandwidth schedule ----
        # A unit: one fp8 tile -> ACT exp + accum_out row sum.
        # B unit: one bf16 DMA of KCH chunks -> DVE schraudolph -> 4 paired
        #         matmuls. Alternating A/B units gives ACT ~160 GB/s of the
        #         ~360 GB/s FIFO, matching its 154 GB/s consumption; leftover
        #         B units drain after region A is fully issued.
        NMM = NCH // 2
        nmm = 0

        a_units = []
        aoff = [0] * G
        for t in range(NTA):
            for g in range(G):
                a_units.append((g, t, aoff[g], ATILES[t]))
                aoff[g] += ATILES[t]

        b_starts = list(range(0, NCH, KCH))

        def emit_a(g, t, off, w):
            rs = slice(g * P, (g + 1) * P)
            xt = apool.tile([P, w], FP8, tag="xt")
            nc.sync.dma_start(xt[:], xa[rs, off:off + w])
            et = epool.tile([P, w], BF16, tag="et")
            idx = g * NTA + t
            nc.scalar.activation(et[:], xt[:], AF.Exp, scale=SCALE,
                                 accum_out=ssum[:, idx:idx + 1])

        def emit_b(ch0):
            nonlocal nmm
            k = min(KCH, NCH - ch0)
            xb = bpool.tile([P, KCH * RPC], BF16, tag="xb")
            nc.sync.dma_start(xb[:, :k * RPC],
                              xbt[:, ch0 * RPC:(ch0 + k) * RPC])
            si = ipool.tile([P, KCH * RPC], I16, tag="si")
            nc.vector.tensor_scalar(si[:, :k * RPC], xb[:, :k * RPC],
                                    S1, S2, ALU.mult, ALU.add)
            for j in range(k // 2):
                bk = banks[nmm % NBANK]
                rhs = si[:, j * 2 * RPC:(j + 1) * 2 * RPC].bitcast(BF16)
                nc.tensor.matmul(bk[:1], ones[:], rhs,
                                 start=(nmm < NBANK),
                                 stop=(nmm >= NMM - NBANK))
                nmm += 1

        # all of region A fits in apool (CA bytes/partition total), so issue
        # it entirely up front: ACT then runs gap-free with no further feed,
        # and every region-B byte drains right behind it, minimizing the
        # B-side tail (last-B -> schraudolph -> matmul -> copies -> osb).
        for au in a_units:
            emit_a(*au)
        for ch0 in b_starts:
            emit_b(ch0)

        # ---- outputs (psum copies split across Vector and Scalar) ----
        sb = spool.tile([1, NBANK * 2 * RPC], F32)
        W = 2 * RPC
        # ACT is idle by now; split psum evacuation across Vector and Scalar
        for kb in range(NBANK):
            dst = sb[:, kb * W:(kb + 1) * W]
            if kb % 2 == 0:
                nc.vector.tensor_copy(dst, banks[kb][:1])
            else:
                nc.scalar.copy(dst, banks[kb][:1])
        nc.sync.dma_start(osb[:, :], sb[:])
        nc.sync.dma_start(osa[:, :], ssum[:])

    nc.compile()
    return nc


def _get_nc():
    if "nc" not in _cache:
        _cache["nc"] = _build()
    return _cache["nc"]


def _shard(preds, labels):
    """Quantize to fp8-e4m3 and build per-core region A/B device layouts."""
    preds = np.ascontiguousarray(preds, dtype=np.float32)
    q = preds.astype(E4M3)
    in_maps = []
    for c in range(N_CORES):
        rows = slice(c * RPC, (c + 1) * RPC)
        qa = np.ascontiguousarray(q[rows, :CA])
        # [256, VB] -> [VB, 256] -> chunks of 128 classes along free dim;
        # bf16 upcast is exact (fp8 values are representable in bf16)
        qb = np.ascontiguousarray(
            q[rows, CA:].T.reshape(NCH, P, RPC).transpose(1, 0, 2)
            .reshape(P, NCH * RPC).astype(ml_dtypes.bfloat16))
        in_maps.append({"xa": qa, "xbt": qb})
    return in_maps


def kernel(preds, labels):
    preds = np.ascontiguousarray(preds, dtype=np.float32)
    labels = np.asarray(labels).astype(np.int64)
    in_maps = _shard(preds, labels)
    nc = _get_nc()
    res = run_bass_kernel_spmd(nc, in_maps, list(range(N_CORES)))

    # device row sums S (all classes, fp8-quantized)
    S = np.empty(B, dtype=np.float64)
    for c in range(N_CORES):
        r = res.results[c]
        sa = np.asarray(r["osa"], np.float64)            # [128, G*NTA]
        sb = np.asarray(r["osb"], np.float64)[0]         # [4*512]
        s_a = np.zeros(RPC)
        for g in range(G):
            s_a[g * P:(g + 1) * P] = sa[:, g * NTA:(g + 1) * NTA].sum(axis=1)
        # each bank holds two 256-col half-sums (paired chunks)
        s_b = sb.reshape(NBANK * 2, RPC).sum(axis=0)
        S[c * RPC:(c + 1) * RPC] = s_a + s_b

    # subtract the device's own target-column contribution (exact simulation)
    idx = np.arange(B)
    tq32 = preds[idx, labels].astype(E4M3).astype(np.float32)
    in_a = labels < CA
    sub = np.empty(B, dtype=np.float64)
    sub[in_a] = np.exp(np.float64(SCALE) * tq32[in_a].astype(np.float64))
    vb = (tq32[~in_a] * np.float32(S1) + np.float32(S2)).astype(np.float32)
    i16 = np.rint(vb.astype(np.float64)).astype(np.int16)
    sub[~in_a] = i16.view(ml_dtypes.bfloat16).astype(np.float64)
    S_others = S - sub

    # numerator from exact f32 targets (reference formula)
    t = preds[idx, labels].astype(np.float64)
    eps = 1e-12
    theta = np.arccos(np.clip(t, -1.0 + eps, 1.0 - eps))
    theta = np.clip(theta, eps, np.pi - eps)
    num = SCALE * (np.cos(theta + 0.5) - 0.35)

    den = np.exp(num) + S_others
    loss = np.mean(np.log(den) - num)
    return np.array(loss, dtype=np.float32)


# revision 17
# speedup vs baseline: 1.0729x; 1.0729x over previous
"""CosArcLoss on 8 TRN2 NeuronCores (Bass/Tile), fp8 two-region pipeline.

Math (reference, f32):
    t_i   = preds[i, labels[i]]
    num_i = 30*(cos(arccos(clip(t_i)) + 0.5) - 0.35)
    S_i   = sum_{j != labels[i]} exp(30*preds[i,j])
    loss  = mean_i( log(exp(num_i) + S_i) - num_i )

Device does all O(B*V) work: sum_j exp(30*q(x_ij)) over fp8-quantized
inputs (tolerance 2e-2 >> fp8 logsumexp bias ~4e-3). Host does the O(B)
epilogue: numerator from exact f32 targets, subtraction of the (exactly
simulated) target-column device contribution, final log/mean.

Per-core layout (256 rows, 32000 classes), split by class:
  region A (classes [0, CA)):  row-major fp8, HWDGE DMA, ScalarE exp
      (scale=30) with free accum_out row-sums. ~0.83 ns/elem on ACT.
      Two small warm-up tiles per row group start the ACT pipeline early.
  region B (classes [CA, V)):  transposed fp8 [class, row] chunks of 128
      classes, SWDGE DMA casts fp8->bf16 in flight (~305 GB/s write-side,
      queue-serialized so few big DMAs), VectorE computes exp via the
      Schraudolph exp2 bit-trick (i16 = rint(x*S1+S2) whose bits ARE
      bf16(exp(30x)), 4x mode ~0.27 ns/elem), TensorE ones-matmul reduces
      pairs of chunks (N=512 = one PSUM bank) into 4 rotating accumulators.
All engines stream concurrently; the split CA/VB balances ACT busy time
against the SWDGE cast-queue drain rate.
"""
import numpy as np
import ml_dtypes
from contextlib import ExitStack

import concourse.bass as bass
import concourse.tile as tile
from concourse import bacc, mybir
from concourse.bass_utils import run_bass_kernel_spmd

B, V = 2048, 32000
N_CORES = 8
RPC = B // N_CORES            # 256 rows per core
P = 128                       # SBUF partitions
G = RPC // P                  # 2 row groups (region A)

CA = 16384                    # classes handled by ACT (region A)
VB = V - CA                   # classes handled by DVE+TensorE (region B)
NCH = VB // P                 # 122 chunks of 128 classes
ATILES = [1024, 3072, 4096, 4096, 4096]  # per-group ACT tile widths
NTA = len(ATILES)
assert sum(ATILES) == CA
KCH = 16                      # chunks per B DMA / DVE schraudolph op
NBANK = 4                     # rotating PSUM accumulators (512 cols each)

SCALE = 30.0
LN2 = float(np.log(2.0))
S1 = 128.0 * SCALE / LN2           # schraudolph slope (bf16 bits / x)
C0 = 0.0564005                     # zero-mean-rel-err offset
S2 = 128.0 * (127.0 - C0)

F32 = mybir.dt.float32
BF16 = mybir.dt.bfloat16
I16 = mybir.dt.int16
FP8 = mybir.dt.float8e4
AF = mybir.ActivationFunctionType
ALU = mybir.AluOpType
E4M3 = ml_dtypes.float8_e4m3

_cache = {}


def _build():
    nc = bacc.Bacc("TRN2", target_bir_lowering=False, debug=False,
                   num_devices=N_CORES)
    xa = nc.dram_tensor("xa", [RPC, CA], FP8, kind="ExternalInput")
    xbt = nc.dram_tensor("xbt", [P, NCH * RPC], BF16, kind="ExternalInput")
    osa = nc.dram_tensor("osa", [P, G * NTA], F32, kind="ExternalOutput")
    osb = nc.dram_tensor("osb", [1, NBANK * 2 * RPC], F32,
                         kind="ExternalOutput")

    with tile.TileContext(nc) as tc, ExitStack() as ctx:
        apool = ctx.enter_context(tc.tile_pool(name="ap", bufs=12))
        bpool = ctx.enter_context(tc.tile_pool(name="bp", bufs=6))
        epool = ctx.enter_context(tc.tile_pool(name="ep", bufs=2))
        ipool = ctx.enter_context(tc.tile_pool(name="ip", bufs=4))
        spool = ctx.enter_context(tc.tile_pool(name="sp", bufs=1))
        psum = ctx.enter_context(tc.tile_pool(name="ps", bufs=1, space="PSUM"))

        ssum = spool.tile([P, G * NTA], F32)
        ones = spool.tile([P, 1], BF16)
        nc.any.memset(ones[:], 1.0)
        banks = [psum.tile([P, 2 * RPC], F32, name=f"bank{k}")
                 for k in range(NBANK)]

        # ---- single HWDGE queue, issue order = bandwidth schedule ----
        # A unit: one fp8 tile -> ACT exp + accum_out row sum.
        # B unit: one bf16 DMA of KCH chunks -> DVE schraudolph -> 4 paired
        #         matmuls. Alternating A/B units gives ACT ~160 GB/s of the
        #         ~360 GB/s FIFO, matching its 154 GB/s consumption; leftover
        #         B units drain after region A is fully issued.
        NMM = NCH // 2
        nmm = 0

        a_units = []
        aoff = [0] * G
        for t in range(NTA):
            for g in range(G):
                a_units.append((g, t, aoff[g], ATILES[t]))
                aoff[g] += ATILES[t]

        b_starts = list(range(0, NCH, KCH))

        def emit_a(g, t, off, w):
            rs = slice(g * P, (g + 1) * P)
            xt = apool.tile([P, w], FP8, tag="xt")
            nc.sync.dma_start(xt[:], xa[rs, off:off + w])
            et = epool.tile([P, w], BF16, tag="et")
            idx = g * NTA + t
            nc.scalar.activation(et[:], xt[:], AF.Exp, scale=SCALE,
                                 accum_out=ssum[:, idx:idx + 1])

        def emit_b(ch0):
            nonlocal nmm
            k = min(KCH, NCH - ch0)
            xb = bpool.tile([P, KCH * RPC], BF16, tag="xb")
            nc.sync.dma_start(xb[:, :k * RPC],
                              xbt[:, ch0 * RPC:(ch0 + k) * RPC])
            si = ipool.tile([P, KCH * RPC], I16, tag="si")
            nc.vector.tensor_scalar(si[:, :k * RPC], xb[:, :k * RPC],
                                    S1, S2, ALU.mult, ALU.add)
            for j in range(k // 2):
                bk = banks[nmm % NBANK]
                rhs = si[:, j * 2 * RPC:(j + 1) * 2 * RPC].bitcast(BF16)
                nc.tensor.matmul(bk[:1], ones[:], rhs,
                                 start=(nmm < NBANK),
                                 stop=(nmm >= NMM - NBANK))
                nmm += 1

        ai = bi = 0
        emit_a(*a_units[ai]); ai += 1       # prime ACT with two tiles
        emit_a(*a_units[ai]); ai += 1
        while ai < len(a_units) or bi < len(b_starts):
            if bi < len(b_starts):
                emit_b(b_starts[bi]); bi += 1
            if ai < len(a_units):
                emit_a(*a_units[ai]); ai += 1

        # ---- outputs (psum copies split across Vector and Scalar) ----
        sb = spool.tile([1, NBANK * 2 * RPC], F32)
        W = 2 * RPC
        # ACT is idle by now; split psum evacuation across Vector and Scalar
        for kb in range(NBANK):
            dst = sb[:, kb * W:(kb + 1) * W]
            if kb % 2 == 0:
                nc.vector.tensor_copy(dst, banks[kb][:1])
            else:
                nc.scalar.copy(dst, banks[kb][:1])
        nc.sync.dma_start(osb[:, :], sb[:])
        nc.sync.dma_start(osa[:, :], ssum[:])

    nc.compile()
    return nc


def _get_nc():
    if "nc" not in _cache:
        _cache["nc"] = _build()
    return _cache["nc"]


def _shard(preds, labels):
    """Quantize to fp8-e4m3 and build per-core region A/B device layouts."""
    preds = np.ascontiguousarray(preds, dtype=np.float32)
    q = preds.astype(E4M3)
    in_maps = []
    for c in range(N_CORES):
        rows = slice(c * RPC, (c + 1) * RPC)
        qa = np.ascontiguousarray(q[rows, :CA])
        # [256, VB] -> [VB, 256] -> chunks of 128 classes along free dim;
        # bf16 upcast is exact (fp8 values are representable in bf16)
        qb = np.ascontiguousarray(
            q[rows, CA:].T.reshape(NCH, P, RPC).transpose(1, 0, 2)
            .reshape(P, NCH * RPC).astype(ml_dtypes.bfloat16))
        in_maps.append({"xa": qa, "xbt": qb})
    return in_maps


def kernel(preds, labels):
    preds = np.ascontiguousarray(preds, dtype=np.float32)
    labels = np.asarray(labels).astype(np.int64)
    in_maps = _shard(preds, labels)
    nc = _get_nc()
    res = run_bass_kernel_spmd(nc, in_maps, list(range(N_CORES)))

    # device row sums S (all classes, fp8-quantized)
    S = np.empty(B, dtype=np.float64)
    for c in range(N_CORES):
        r = res.results[c]
        sa = np.asarray(r["osa"], np.float64)            # [128, G*NTA]
        sb = np.asarray(r["osb"], np.float64)[0]         # [4*512]
        s_a = np.zeros(RPC)
        for g in range(G):
            s_a[g * P:(g + 1) * P] = sa[:, g * NTA:(g + 1) * NTA].sum(axis=1)
        # each bank holds two 256-col half-sums (paired chunks)
        s_b = sb.reshape(NBANK * 2, RPC).sum(axis=0)
        S[c * RPC:(c + 1) * RPC] = s_a + s_b

    # subtract the device's own target-column contribution (exact simulation)
    idx = np.arange(B)
    tq32 = preds[idx, labels].astype(E4M3).astype(np.float32)
    in_a = labels < CA
    sub = np.empty(B, dtype=np.float64)
    sub[in_a] = np.exp(np.float64(SCALE) * tq32[in_a].astype(np.float64))
    vb = (tq32[~in_a] * np.float32(S1) + np.float32(S2)).astype(np.float32)
    i16 = np.rint(vb.astype(np.float64)).astype(np.int16)
    sub[~in_a] = i16.view(ml_dtypes.bfloat16).astype(np.float64)
    S_others = S - sub

    # numerator from exact f32 targets (reference formula)
    t = preds[idx, labels].astype(np.float64)
    eps = 1e-12
    theta = np.arccos(np.clip(t, -1.0 + eps, 1.0 - eps))
    theta = np.clip(theta, eps, np.pi - eps)
    num = SCALE * (np.cos(theta + 0.5) - 0.35)

    den = np.exp(num) + S_others
    loss = np.mean(np.log(den) - num)
    return np.array(loss, dtype=np.float32)


# revision 18
# speedup vs baseline: 1.0900x; 1.0159x over previous
"""CosArcLoss on 8 TRN2 NeuronCores (Bass/Tile), fp8 two-region pipeline.

Math (reference, f32):
    t_i   = preds[i, labels[i]]
    num_i = 30*(cos(arccos(clip(t_i)) + 0.5) - 0.35)
    S_i   = sum_{j != labels[i]} exp(30*preds[i,j])
    loss  = mean_i( log(exp(num_i) + S_i) - num_i )

Device does all O(B*V) work: sum_j exp(30*q(x_ij)) over fp8-quantized
inputs (tolerance 2e-2 >> fp8 logsumexp bias ~4e-3). Host does the O(B)
epilogue: numerator from exact f32 targets, subtraction of the (exactly
simulated) target-column device contribution, final log/mean.

Per-core layout (256 rows, 32000 classes), split by class:
  region A (classes [0, CA)):  row-major fp8, HWDGE DMA, ScalarE exp
      (scale=30) with free accum_out row-sums. ~0.83 ns/elem on ACT.
      Two small warm-up tiles per row group start the ACT pipeline early.
  region B (classes [CA, V)):  transposed fp8 [class, row] chunks of 128
      classes, SWDGE DMA casts fp8->bf16 in flight (~305 GB/s write-side,
      queue-serialized so few big DMAs), VectorE computes exp via the
      Schraudolph exp2 bit-trick (i16 = rint(x*S1+S2) whose bits ARE
      bf16(exp(30x)), 4x mode ~0.27 ns/elem), TensorE ones-matmul reduces
      pairs of chunks (N=512 = one PSUM bank) into 4 rotating accumulators.
All engines stream concurrently; the split CA/VB balances ACT busy time
against the SWDGE cast-queue drain rate.
"""
import numpy as np
import ml_dtypes
from contextlib import ExitStack

import concourse.bass as bass
import concourse.tile as tile
from concourse import bacc, mybir
from concourse.bass_utils import run_bass_kernel_spmd

B, V = 2048, 32000
N_CORES = 8
RPC = B // N_CORES            # 256 rows per core
P = 128                       # SBUF partitions
G = RPC // P                  # 2 row groups (region A)

CA = 15104                    # classes handled by ACT (region A)
VB = V - CA                   # classes handled by DVE+TensorE (region B)
NCH = VB // P                 # 132 chunks of 128 classes
ATILES = [1888] * 8           # per-group ACT tile widths
NTA = len(ATILES)
assert sum(ATILES) == CA
KCH = 16                      # chunks per B DMA / DVE schraudolph op
NBANK = 4                     # rotating PSUM accumulators (512 cols each)

SCALE = 30.0
LN2 = float(np.log(2.0))
S1 = 128.0 * SCALE / LN2           # schraudolph slope (bf16 bits / x)
C0 = 0.0564005                     # zero-mean-rel-err offset
S2 = 128.0 * (127.0 - C0)

F32 = mybir.dt.float32
BF16 = mybir.dt.bfloat16
I16 = mybir.dt.int16
FP8 = mybir.dt.float8e4
AF = mybir.ActivationFunctionType
ALU = mybir.AluOpType
E4M3 = ml_dtypes.float8_e4m3

_cache = {}


def _build():
    nc = bacc.Bacc("TRN2", target_bir_lowering=False, debug=False,
                   num_devices=N_CORES)
    xa = nc.dram_tensor("xa", [RPC, CA], FP8, kind="ExternalInput")
    xbt = nc.dram_tensor("xbt", [P, NCH * RPC], FP8, kind="ExternalInput")
    osa = nc.dram_tensor("osa", [P, G * NTA], F32, kind="ExternalOutput")
    osb = nc.dram_tensor("osb", [1, NBANK * 2 * RPC], F32,
                         kind="ExternalOutput")

    with tile.TileContext(nc) as tc, ExitStack() as ctx:
        apool = ctx.enter_context(tc.tile_pool(name="ap", bufs=12))
        bpool = ctx.enter_context(tc.tile_pool(name="bp", bufs=3))
        epool = ctx.enter_context(tc.tile_pool(name="ep", bufs=2))
        ipool = ctx.enter_context(tc.tile_pool(name="ip", bufs=4))
        spool = ctx.enter_context(tc.tile_pool(name="sp", bufs=1))
        psum = ctx.enter_context(tc.tile_pool(name="ps", bufs=1, space="PSUM"))

        ssum = spool.tile([P, G * NTA], F32)
        ones = spool.tile([P, 1], BF16)
        nc.any.memset(ones[:], 1.0)
        banks = [psum.tile([P, 2 * RPC], F32, name=f"bank{k}")
                 for k in range(NBANK)]

        # ---- single HWDGE queue, issue order = bandwidth schedule ----
        # A unit: one fp8 tile -> ACT exp + accum_out row sum.
        # B unit: one bf16 DMA of KCH chunks -> DVE schraudolph -> 4 paired
        #         matmuls. Alternating A/B units gives ACT ~160 GB/s of the
        #         ~360 GB/s FIFO, matching its 154 GB/s consumption; leftover
        #         B units drain after region A is fully issued.
        NMM = NCH // 2
        nmm = 0

        a_units = []
        aoff = [0] * G
        for t in range(NTA):
            for g in range(G):
                a_units.append((g, t, aoff[g], ATILES[t]))
                aoff[g] += ATILES[t]

        b_starts = list(range(0, NCH, KCH))

        def emit_a(g, t, off, w):
            rs = slice(g * P, (g + 1) * P)
            xt = apool.tile([P, w], FP8, tag="xt")
            nc.sync.dma_start(xt[:], xa[rs, off:off + w])
            et = epool.tile([P, w], BF16, tag="et")
            idx = g * NTA + t
            nc.scalar.activation(et[:], xt[:], AF.Exp, scale=SCALE,
                                 accum_out=ssum[:, idx:idx + 1])

        def emit_b(ch0):
            nonlocal nmm
            k = min(KCH, NCH - ch0)
            xb = bpool.tile([P, KCH * RPC], BF16, tag="xb")
            nc.gpsimd.dma_start(xb[:, :k * RPC],
                                xbt[:, ch0 * RPC:(ch0 + k) * RPC])
            si = ipool.tile([P, KCH * RPC], I16, tag="si")
            nc.vector.tensor_scalar(si[:, :k * RPC], xb[:, :k * RPC],
                                    S1, S2, ALU.mult, ALU.add)
            for j in range(k // 2):
                bk = banks[nmm % NBANK]
                rhs = si[:, j * 2 * RPC:(j + 1) * 2 * RPC].bitcast(BF16)
                nc.tensor.matmul(bk[:1], ones[:], rhs,
                                 start=(nmm < NBANK),
                                 stop=(nmm >= NMM - NBANK))
                nmm += 1

        for au in a_units:
            emit_a(*au)
        for ch0 in b_starts:
            emit_b(ch0)

        # ---- outputs (psum copies split across Vector and Scalar) ----
        sb = spool.tile([1, NBANK * 2 * RPC], F32)
        W = 2 * RPC
        # ACT is idle by now; split psum evacuation across Vector and Scalar
        for kb in range(NBANK):
            dst = sb[:, kb * W:(kb + 1) * W]
            if kb % 2 == 0:
                nc.vector.tensor_copy(dst, banks[kb][:1])
            else:
                nc.scalar.copy(dst, banks[kb][:1])
        nc.sync.dma_start(osb[:, :], sb[:])
        nc.sync.dma_start(osa[:, :], ssum[:])

    nc.compile()
    return nc


def _get_nc():
    if "nc" not in _cache:
        _cache["nc"] = _build()
    return _cache["nc"]


def _shard(preds, labels):
    """Quantize to fp8-e4m3 and build per-core region A/B device layouts."""
    preds = np.ascontiguousarray(preds, dtype=np.float32)
    q = preds.astype(E4M3)
    in_maps = []
    for c in range(N_CORES):
        rows = slice(c * RPC, (c + 1) * RPC)
        qa = np.ascontiguousarray(q[rows, :CA])
        # [256, VB] -> [VB, 256] -> chunks of 128 classes along free dim
        qb = np.ascontiguousarray(
            q[rows, CA:].T.reshape(NCH, P, RPC).transpose(1, 0, 2)
            .reshape(P, NCH * RPC))
        in_maps.append({"xa": qa, "xbt": qb})
    return in_maps


def kernel(preds, labels):
    preds = np.ascontiguousarray(preds, dtype=np.float32)
    labels = np.asarray(labels).astype(np.int64)
    in_maps = _shard(preds, labels)
    nc = _get_nc()
    res = run_bass_kernel_spmd(nc, in_maps, list(range(N_CORES)))

    # device row sums S (all classes, fp8-quantized)
    S = np.empty(B, dtype=np.float64)
    for c in range(N_CORES):
        r = res.results[c]
        sa = np.asarray(r["osa"], np.float64)            # [128, G*NTA]
        sb = np.asarray(r["osb"], np.float64)[0]         # [4*512]
        s_a = np.zeros(RPC)
        for g in range(G):
            s_a[g * P:(g + 1) * P] = sa[:, g * NTA:(g + 1) * NTA].sum(axis=1)
        # each bank holds two 256-col half-sums (paired chunks)
        s_b = sb.reshape(NBANK * 2, RPC).sum(axis=0)
        S[c * RPC:(c + 1) * RPC] = s_a + s_b

    # subtract the device's own target-column contribution (exact simulation)
    idx = np.arange(B)
    tq32 = preds[idx, labels].astype(E4M3).astype(np.float32)
    in_a = labels < CA
    sub = np.empty(B, dtype=np.float64)
    sub[in_a] = np.exp(np.float64(SCALE) * tq32[in_a].astype(np.float64))
    vb = (tq32[~in_a] * np.float32(S1) + np.float32(S2)).astype(np.float32)
    i16 = np.rint(vb.astype(np.float64)).astype(np.int16)
    sub[~in_a] = i16.view(ml_dtypes.bfloat16).astype(np.float64)
    S_others = S - sub

    # numerator from exact f32 targets (reference formula)
    t = preds[idx, labels].astype(np.float64)
    eps = 1e-12
    theta = np.arccos(np.clip(t, -1.0 + eps, 1.0 - eps))
    theta = np.clip(theta, eps, np.pi - eps)
    num = SCALE * (np.cos(theta + 0.5) - 0.35)

    den = np.exp(num) + S_others
    loss = np.mean(np.log(den) - num)
    return np.array(loss, dtype=np.float32)


# revision 19
# speedup vs baseline: 1.1606x; 1.0648x over previous
"""CosArcLoss on 8 TRN2 NeuronCores (Bass/Tile), fp8 two-region pipeline.

Math (reference, f32):
    t_i   = preds[i, labels[i]]
    num_i = 30*(cos(arccos(clip(t_i)) + 0.5) - 0.35)
    S_i   = sum_{j != labels[i]} exp(30*preds[i,j])
    loss  = mean_i( log(exp(num_i) + S_i) - num_i )

Device does all O(B*V) work: sum_j exp(30*q(x_ij)) over fp8-quantized
inputs (tolerance 2e-2 >> fp8 logsumexp bias ~4e-3). Host does the O(B)
epilogue: numerator from exact f32 targets, subtraction of the (exactly
simulated) target-column device contribution, final log/mean.

Per-core layout (256 rows, 32000 classes), split by class:
  region A (classes [0, CA)):  row-major fp8, HWDGE DMA, ScalarE exp
      (scale=30) with free accum_out row-sums. ~0.83 ns/elem on ACT.
      Two small warm-up tiles per row group start the ACT pipeline early.
  region B (classes [CA, V)):  transposed fp8 [class, row] chunks of 128
      classes, SWDGE DMA casts fp8->bf16 in flight (~305 GB/s write-side,
      queue-serialized so few big DMAs), VectorE computes exp via the
      Schraudolph exp2 bit-trick (i16 = rint(x*S1+S2) whose bits ARE
      bf16(exp(30x)), 4x mode ~0.27 ns/elem), TensorE ones-matmul reduces
      pairs of chunks (N=512 = one PSUM bank) into 4 rotating accumulators.
All engines stream concurrently; the split CA/VB balances ACT busy time
against the SWDGE cast-queue drain rate.
"""
import numpy as np
import ml_dtypes
from contextlib import ExitStack

import concourse.bass as bass
import concourse.tile as tile
from concourse import bacc, mybir
from concourse.bass_utils import run_bass_kernel_spmd

B, V = 2048, 32000
N_CORES = 8
RPC = B // N_CORES            # 256 rows per core
P = 128                       # SBUF partitions
G = RPC // P                  # 2 row groups (region A)

CA = 14336                    # classes handled by ACT (region A)
VB = V - CA                   # classes handled by DVE+TensorE (region B)
NCH = VB // P                 # 138 chunks of 128 classes
ATILES = [1792] * 8           # per-group ACT tile widths
NTA = len(ATILES)
assert sum(ATILES) == CA
KCH = 16                      # chunks per B DMA / DVE schraudolph op
NBANK = 4                     # rotating PSUM accumulators (512 cols each)

SCALE = 30.0
LN2 = float(np.log(2.0))
S1 = 128.0 * SCALE / LN2           # schraudolph slope (bf16 bits / x)
C0 = 0.0564005                     # zero-mean-rel-err offset
S2 = 128.0 * (127.0 - C0)

F32 = mybir.dt.float32
BF16 = mybir.dt.bfloat16
I16 = mybir.dt.int16
FP8 = mybir.dt.float8e4
AF = mybir.ActivationFunctionType
ALU = mybir.AluOpType
E4M3 = ml_dtypes.float8_e4m3

_cache = {}


def _build():
    nc = bacc.Bacc("TRN2", target_bir_lowering=False, debug=False,
                   num_devices=N_CORES)
    xa = nc.dram_tensor("xa", [RPC, CA], FP8, kind="ExternalInput")
    xbt = nc.dram_tensor("xbt", [P, NCH * RPC], FP8, kind="ExternalInput")
    osa = nc.dram_tensor("osa", [P, G * NTA], F32, kind="ExternalOutput")
    osb = nc.dram_tensor("osb", [1, NBANK * 2 * RPC], F32,
                         kind="ExternalOutput")

    with tile.TileContext(nc) as tc, ExitStack() as ctx:
        apool = ctx.enter_context(tc.tile_pool(name="ap", bufs=16))
        bpool = ctx.enter_context(tc.tile_pool(name="bp", bufs=3))
        epool = ctx.enter_context(tc.tile_pool(name="ep", bufs=2))
        ipool = ctx.enter_context(tc.tile_pool(name="ip", bufs=4))
        spool = ctx.enter_context(tc.tile_pool(name="sp", bufs=1))
        psum = ctx.enter_context(tc.tile_pool(name="ps", bufs=1, space="PSUM"))

        ssum = spool.tile([P, G * NTA], F32)
        ones = spool.tile([P, 1], BF16)
        nc.any.memset(ones[:], 1.0)
        banks = [psum.tile([P, 2 * RPC], F32, name=f"bank{k}")
                 for k in range(NBANK)]

        # ---- single HWDGE queue, issue order = bandwidth schedule ----
        # A unit: one fp8 tile -> ACT exp + accum_out row sum.
        # B unit: one bf16 DMA of KCH chunks -> DVE schraudolph -> 4 paired
        #         matmuls. Alternating A/B units gives ACT ~160 GB/s of the
        #         ~360 GB/s FIFO, matching its 154 GB/s consumption; leftover
        #         B units drain after region A is fully issued.
        NMM = NCH // 2
        nmm = 0

        a_units = []
        aoff = [0] * G
        for t in range(NTA):
            for g in range(G):
                a_units.append((g, t, aoff[g], ATILES[t]))
                aoff[g] += ATILES[t]

        b_starts = list(range(0, NCH, KCH))

        def emit_a(g, t, off, w):
            rs = slice(g * P, (g + 1) * P)
            xt = apool.tile([P, w], FP8, tag="xt")
            nc.sync.dma_start(xt[:], xa[rs, off:off + w])
            et = epool.tile([P, w], BF16, tag="et")
            idx = g * NTA + t
            nc.scalar.activation(et[:], xt[:], AF.Exp, scale=SCALE,
                                 accum_out=ssum[:, idx:idx + 1])

        def emit_b(ch0):
            nonlocal nmm
            k = min(KCH, NCH - ch0)
            xb = bpool.tile([P, KCH * RPC], BF16, tag="xb")
            nc.gpsimd.dma_start(xb[:, :k * RPC],
                                xbt[:, ch0 * RPC:(ch0 + k) * RPC])
            si = ipool.tile([P, KCH * RPC], I16, tag="si")
            nc.vector.tensor_scalar(si[:, :k * RPC], xb[:, :k * RPC],
                                    S1, S2, ALU.mult, ALU.add)
            for j in range(k // 2):
                bk = banks[nmm % NBANK]
                rhs = si[:, j * 2 * RPC:(j + 1) * 2 * RPC].bitcast(BF16)
                nc.tensor.matmul(bk[:1], ones[:], rhs,
                                 start=(nmm < NBANK),
                                 stop=(nmm >= NMM - NBANK))
                nmm += 1

        for au in a_units:
            emit_a(*au)
        for ch0 in b_starts:
            emit_b(ch0)

        # ---- outputs (psum copies split across Vector and Scalar) ----
        sb = spool.tile([1, NBANK * 2 * RPC], F32)
        W = 2 * RPC
        for kb in range(NBANK):
            dst = sb[:, kb * W:(kb + 1) * W]
            nc.vector.tensor_copy(dst, banks[kb][:1])
        nc.sync.dma_start(osb[:, :], sb[:])
        nc.sync.dma_start(osa[:, :], ssum[:])

    nc.compile()
    return nc


def _get_nc():
    if "nc" not in _cache:
        _cache["nc"] = _build()
    return _cache["nc"]


def _shard(preds, labels):
    """Quantize to fp8-e4m3 and build per-core region A/B device layouts."""
    preds = np.ascontiguousarray(preds, dtype=np.float32)
    q = preds.astype(E4M3)
    in_maps = []
    for c in range(N_CORES):
        rows = slice(c * RPC, (c + 1) * RPC)
        qa = np.ascontiguousarray(q[rows, :CA])
        # [256, VB] -> [VB, 256] -> chunks of 128 classes along free dim
        qb = np.ascontiguousarray(
            q[rows, CA:].T.reshape(NCH, P, RPC).transpose(1, 0, 2)
            .reshape(P, NCH * RPC))
        in_maps.append({"xa": qa, "xbt": qb})
    return in_maps


def kernel(preds, labels):
    preds = np.ascontiguousarray(preds, dtype=np.float32)
    labels = np.asarray(labels).astype(np.int64)
    in_maps = _shard(preds, labels)
    nc = _get_nc()
    res = run_bass_kernel_spmd(nc, in_maps, list(range(N_CORES)))

    # device row sums S (all classes, fp8-quantized)
    S = np.empty(B, dtype=np.float64)
    for c in range(N_CORES):
        r = res.results[c]
        sa = np.asarray(r["osa"], np.float64)            # [128, G*NTA]
        sb = np.asarray(r["osb"], np.float64)[0]         # [4*512]
        s_a = np.zeros(RPC)
        for g in range(G):
            s_a[g * P:(g + 1) * P] = sa[:, g * NTA:(g + 1) * NTA].sum(axis=1)
        # each bank holds two 256-col half-sums (paired chunks)
        s_b = sb.reshape(NBANK * 2, RPC).sum(axis=0)
        S[c * RPC:(c + 1) * RPC] = s_a + s_b

    # subtract the device's own target-column contribution (exact simulation)
    idx = np.arange(B)
    tq32 = preds[idx, labels].astype(E4M3).astype(np.float32)
    in_a = labels < CA
    sub = np.empty(B, dtype=np.float64)
    sub[in_a] = np.exp(np.float64(SCALE) * tq32[in_a].astype(np.float64))
    vb = (tq32[~in_a] * np.float32(S1) + np.float32(S2)).astype(np.float32)
    i16 = np.rint(vb.astype(np.float64)).astype(np.int16)
    sub[~in_a] = i16.view(ml_dtypes.bfloat16).astype(np.float64)
    S_others = S - sub

    # numerator from exact f32 targets (reference formula)
    t = preds[idx, labels].astype(np.float64)
    eps = 1e-12
    theta = np.arccos(np.clip(t, -1.0 + eps, 1.0 - eps))
    theta = np.clip(theta, eps, np.pi - eps)
    num = SCALE * (np.cos(theta + 0.5) - 0.35)

    den = np.exp(num) + S_others
    loss = np.mean(np.log(den) - num)
    return np.array(loss, dtype=np.float32)


# revision 20
# speedup vs baseline: 1.1641x; 1.0030x over previous
"""CosArcLoss on 8 TRN2 NeuronCores (Bass/Tile), fp8 two-region pipeline.

Math (reference, f32):
    t_i   = preds[i, labels[i]]
    num_i = 30*(cos(arccos(clip(t_i)) + 0.5) - 0.35)
    S_i   = sum_{j != labels[i]} exp(30*preds[i,j])
    loss  = mean_i( log(exp(num_i) + S_i) - num_i )

Device does all O(B*V) work: sum_j exp(30*q(x_ij)) over fp8-quantized
inputs (tolerance 2e-2 >> fp8 logsumexp bias ~4e-3). Host does the O(B)
epilogue: numerator from exact f32 targets, subtraction of the (exactly
simulated) target-column device contribution, final log/mean.

Per-core layout (256 rows, 32000 classes), split by class:
  region A (classes [0, CA)):  row-major fp8, HWDGE DMA, ScalarE exp
      (scale=30) with free accum_out row-sums. ~0.83 ns/elem on ACT.
      Two small warm-up tiles per row group start the ACT pipeline early.
  region B (classes [CA, V)):  transposed fp8 [class, row] chunks of 128
      classes, SWDGE DMA casts fp8->bf16 in flight (~305 GB/s write-side,
      queue-serialized so few big DMAs), VectorE computes exp via the
      Schraudolph exp2 bit-trick (i16 = rint(x*S1+S2) whose bits ARE
      bf16(exp(30x)), 4x mode ~0.27 ns/elem), TensorE ones-matmul reduces
      pairs of chunks (N=512 = one PSUM bank) into 4 rotating accumulators.
All engines stream concurrently; the split CA/VB balances ACT busy time
against the SWDGE cast-queue drain rate.
"""
import numpy as np
import ml_dtypes
from contextlib import ExitStack

import concourse.bass as bass
import concourse.tile as tile
from concourse import bacc, mybir
from concourse.bass_utils import run_bass_kernel_spmd

B, V = 2048, 32000
N_CORES = 8
RPC = B // N_CORES            # 256 rows per core
P = 128                       # SBUF partitions
G = RPC // P                  # 2 row groups (region A)

CA = 13824                    # classes handled by ACT (region A)
VB = V - CA                   # classes handled by DVE+TensorE (region B)
NCH = VB // P                 # 142 chunks of 128 classes
ATILES = [1728] * 8           # per-group ACT tile widths
NTA = len(ATILES)
assert sum(ATILES) == CA
KCH = 16                      # chunks per B DMA / DVE schraudolph op
NBANK = 4                     # rotating PSUM accumulators (512 cols each)

SCALE = 30.0
LN2 = float(np.log(2.0))
S1 = 128.0 * SCALE / LN2           # schraudolph slope (bf16 bits / x)
C0 = 0.0564005                     # zero-mean-rel-err offset
S2 = 128.0 * (127.0 - C0)

F32 = mybir.dt.float32
BF16 = mybir.dt.bfloat16
I16 = mybir.dt.int16
FP8 = mybir.dt.float8e4
AF = mybir.ActivationFunctionType
ALU = mybir.AluOpType
E4M3 = ml_dtypes.float8_e4m3

_cache = {}


def _build():
    nc = bacc.Bacc("TRN2", target_bir_lowering=False, debug=False,
                   num_devices=N_CORES)
    xa = nc.dram_tensor("xa", [RPC, CA], FP8, kind="ExternalInput")
    xbt = nc.dram_tensor("xbt", [P, NCH * RPC], FP8, kind="ExternalInput")
    osa = nc.dram_tensor("osa", [P, G * NTA], F32, kind="ExternalOutput")
    osb = nc.dram_tensor("osb", [1, NBANK * 2 * RPC], F32,
                         kind="ExternalOutput")

    with tile.TileContext(nc) as tc, ExitStack() as ctx:
        apool = ctx.enter_context(tc.tile_pool(name="ap", bufs=16))
        bpool = ctx.enter_context(tc.tile_pool(name="bp", bufs=3))
        epool = ctx.enter_context(tc.tile_pool(name="ep", bufs=2))
        ipool = ctx.enter_context(tc.tile_pool(name="ip", bufs=4))
        spool = ctx.enter_context(tc.tile_pool(name="sp", bufs=1))
        psum = ctx.enter_context(tc.tile_pool(name="ps", bufs=1, space="PSUM"))

        ssum = spool.tile([P, G * NTA], F32)
        ones = spool.tile([P, 1], BF16)
        nc.any.memset(ones[:], 1.0)
        banks = [psum.tile([P, 2 * RPC], F32, name=f"bank{k}")
                 for k in range(NBANK)]

        # ---- single HWDGE queue, issue order = bandwidth schedule ----
        # A unit: one fp8 tile -> ACT exp + accum_out row sum.
        # B unit: one bf16 DMA of KCH chunks -> DVE schraudolph -> 4 paired
        #         matmuls. Alternating A/B units gives ACT ~160 GB/s of the
        #         ~360 GB/s FIFO, matching its 154 GB/s consumption; leftover
        #         B units drain after region A is fully issued.
        NMM = NCH // 2
        BLK = (NMM + NBANK - 1) // NBANK   # matmuls per psum bank (block)
        nmm = 0

        a_units = []
        aoff = [0] * G
        for t in range(NTA):
            for g in range(G):
                a_units.append((g, t, aoff[g], ATILES[t]))
                aoff[g] += ATILES[t]

        b_starts = list(range(0, NCH, KCH))

        def emit_a(g, t, off, w):
            rs = slice(g * P, (g + 1) * P)
            xt = apool.tile([P, w], FP8, tag="xt")
            nc.sync.dma_start(xt[:], xa[rs, off:off + w])
            et = epool.tile([P, w], BF16, tag="et")
            idx = g * NTA + t
            nc.scalar.activation(et[:], xt[:], AF.Exp, scale=SCALE,
                                 accum_out=ssum[:, idx:idx + 1])

        def emit_b(ch0):
            nonlocal nmm
            k = min(KCH, NCH - ch0)
            xb = bpool.tile([P, KCH * RPC], BF16, tag="xb")
            nc.gpsimd.dma_start(xb[:, :k * RPC],
                                xbt[:, ch0 * RPC:(ch0 + k) * RPC])
            si = ipool.tile([P, KCH * RPC], I16, tag="si")
            nc.vector.tensor_scalar(si[:, :k * RPC], xb[:, :k * RPC],
                                    S1, S2, ALU.mult, ALU.add)
            for j in range(k // 2):
                kb = min(nmm // BLK, NBANK - 1)
                rhs = si[:, j * 2 * RPC:(j + 1) * 2 * RPC].bitcast(BF16)
                nc.tensor.matmul(banks[kb][:1], ones[:], rhs,
                                 start=(nmm % BLK == 0),
                                 stop=((nmm + 1) % BLK == 0 or nmm == NMM - 1))
                nmm += 1

        sb = spool.tile([1, NBANK * 2 * RPC], F32)
        W = 2 * RPC
        copied = [False] * NBANK

        for au in a_units:
            emit_a(*au)
        for ch0 in b_starts:
            emit_b(ch0)
            # evacuate any bank whose accumulation block just finished
            for kb in range(NBANK):
                if not copied[kb] and nmm > 0 and (kb + 1) * BLK <= nmm:
                    nc.vector.tensor_copy(sb[:, kb * W:(kb + 1) * W],
                                          banks[kb][:1])
                    copied[kb] = True
        for kb in range(NBANK):
            if not copied[kb]:
                nc.vector.tensor_copy(sb[:, kb * W:(kb + 1) * W],
                                      banks[kb][:1])
        nc.sync.dma_start(osb[:, :], sb[:])
        nc.sync.dma_start(osa[:, :], ssum[:])

    nc.compile()
    return nc


def _get_nc():
    if "nc" not in _cache:
        _cache["nc"] = _build()
    return _cache["nc"]


def _shard(preds, labels):
    """Quantize to fp8-e4m3 and build per-core region A/B device layouts."""
    preds = np.ascontiguousarray(preds, dtype=np.float32)
    q = preds.astype(E4M3)
    in_maps = []
    for c in range(N_CORES):
        rows = slice(c * RPC, (c + 1) * RPC)
        qa = np.ascontiguousarray(q[rows, :CA])
        # [256, VB] -> [VB, 256] -> chunks of 128 classes along free dim
        qb = np.ascontiguousarray(
            q[rows, CA:].T.reshape(NCH, P, RPC).transpose(1, 0, 2)
            .reshape(P, NCH * RPC))
        in_maps.append({"xa": qa, "xbt": qb})
    return in_maps


def kernel(preds, labels):
    preds = np.ascontiguousarray(preds, dtype=np.float32)
    labels = np.asarray(labels).astype(np.int64)
    in_maps = _shard(preds, labels)
    nc = _get_nc()
    res = run_bass_kernel_spmd(nc, in_maps, list(range(N_CORES)))

    # device row sums S (all classes, fp8-quantized)
    S = np.empty(B, dtype=np.float64)
    for c in range(N_CORES):
        r = res.results[c]
        sa = np.asarray(r["osa"], np.float64)            # [128, G*NTA]
        sb = np.asarray(r["osb"], np.float64)[0]         # [4*512]
        s_a = np.zeros(RPC)
        for g in range(G):
            s_a[g * P:(g + 1) * P] = sa[:, g * NTA:(g + 1) * NTA].sum(axis=1)
        # each bank holds two 256-col half-sums (paired chunks)
        s_b = sb.reshape(NBANK * 2, RPC).sum(axis=0)
        S[c * RPC:(c + 1) * RPC] = s_a + s_b

    # subtract the device's own target-column contribution (exact simulation)
    idx = np.arange(B)
    tq32 = preds[idx, labels].astype(E4M3).astype(np.float32)
    in_a = labels < CA
    sub = np.empty(B, dtype=np.float64)
    sub[in_a] = np.exp(np.float64(SCALE) * tq32[in_a].astype(np.float64))
    vb = (tq32[~in_a] * np.float32(S1) + np.float32(S2)).astype(np.float32)
    i16 = np.rint(vb.astype(np.float64)).astype(np.int16)
    sub[~in_a] = i16.view(ml_dtypes.bfloat16).astype(np.float64)
    S_others = S - sub

    # numerator from exact f32 targets (reference formula)
    t = preds[idx, labels].astype(np.float64)
    eps = 1e-12
    theta = np.arccos(np.clip(t, -1.0 + eps, 1.0 - eps))
    theta = np.clip(theta, eps, np.pi - eps)
    num = SCALE * (np.cos(theta + 0.5) - 0.35)

    den = np.exp(num) + S_others
    loss = np.mean(np.log(den) - num)
    return np.array(loss, dtype=np.float32)
